# revision 1
# baseline (speedup 1.0000x reference)
"""2-layer GCN encoder on 8 TRN2 NeuronCores.

Strategy (node/graph parallel, per sharding hint):
  - shard nodes (output rows) across 8 cores: core c owns cols [c*6250, (c+1)*6250)
  - normalize:  out = D^-1/2 (A+I) D^-1/2 h  ==>  pre/post row scaling by dinv,
    so the per-edge norm multiply disappears: s[c] = sum_{e: col=c} g[row_e],
    g = dinv * (x @ W), out = dinv * s + b.
  - per core: z-shard matmul -> AllGather g1 (fp16) -> edge gather (dma_gather)
    + one-hot S matmul scatter-add in PSUM -> relu -> local @W2 -> AllGather g2
    -> second edge gather/scatter -> output shard.
  - all index prep (sort edges by target group, halo routing, padding, int16
    wrapped index tables) happens on host inside kernel().
"""
import numpy as np

LAST_NC = None
LAST_IN_MAPS = None
from contextlib import ExitStack

import concourse.bass as bass
import concourse.bacc as bacc
import concourse.mybir as mybir
from concourse.bass_utils import run_bass_kernel_spmd
from concourse.tile import TileContext
from concourse.library_config import mlp

# problem constants (hardcoded per contract)
N, E, F, H, FO = 50000, 800000, 128, 128, 64
NCORES = 8
P = N // NCORES            # 6250 own nodes per core
NGRP = 49                  # col groups of 128 (6272 padded)
PPAD = NGRP * 128          # 6272
TBL = NCORES * PPAD        # 50176 rows in gathered tables
HALF = TBL // 2            # 25088 (< int16 max after offset)
GB = 8                     # chunks per dma_gather (1024 indices; >1024 hangs SWDGE)
SB = 8                     # chunks per batched S-generation

fp32, fp16, i16 = mybir.dt.float32, mybir.dt.float16, mybir.dt.int16


def _wrap_idx(idx, nch):
    """int16 index table in the [16-partition wrap x8 replicate] layout."""
    n = nch * 128
    w = np.zeros((16, n // 16), np.int16)
    pos = np.arange(n)
    w[pos % 16, pos // 16] = idx
    return np.tile(w, (8, 1))


def _prep(edge_index):
    """Host-side graph prep. Returns per-core tables + compile-time chunk plan."""
    row = np.asarray(edge_index[0], np.int64)
    col = np.asarray(edge_index[1], np.int64)
    deg = np.bincount(col, minlength=N).astype(np.float64) + 1.0
    dinv = (1.0 / np.sqrt(deg)).astype(np.float32)

    loops = np.arange(N, dtype=np.int64)
    rows = np.concatenate([row, loops])
    cols = np.concatenate([col, loops])
    core = cols // P
    lcol = cols - core * P
    grp = lcol >> 7
    lc128 = lcol & 127
    trow = (rows // P) * PPAD + (rows % P)   # row index in gathered table
    half = (trow >= HALF).astype(np.int64)
    idx16 = trow - half * HALF               # < 25088 fits int16

    # counts[core, half, grp]
    key = (core * 2 + half) * NGRP + grp
    cnt = np.bincount(key, minlength=NCORES * 2 * NGRP).reshape(NCORES, 2, NGRP)
    nch_grp = -(-cnt.max(axis=0) // 128)     # [2, NGRP] chunks per (half, grp)
    nch_s = [int(nch_grp[h].sum()) for h in (0, 1)]
    # pad each stream's chunk count to a multiple of GB (tail chunks -> grp 48)
    import math
    _q = math.lcm(GB, SB)
    tail = [(-nch_s[h]) % _q for h in (0, 1)]
    nch_grp_pad = nch_grp.copy()
    nch_grp_pad[0, NGRP - 1] += tail[0]
    nch_grp_pad[1, NGRP - 1] += tail[1]
    nch_s = [nch_s[h] + tail[h] for h in (0, 1)]

    # per-core edge tables
    order = np.lexsort((grp, half, core))    # sort by core, half, grp (stable)
    rows_s, idx_s, lc_s, grp_s, half_s, core_s = (
        a[order] for a in (rows, idx16, lc128, grp, half, core))

    per_core = []
    for c in range(NCORES):
        sel = core_s == c
        ih, il, ig = idx_s[sel], lc_s[sel], grp_s[sel]
        hh = half_s[sel]
        tabs = {}
        for h in (0, 1):
            m = hh == h
            gi, gl, gg = ih[m], il[m], ig[m]
            nidx = nch_s[h] * 128
            idx_full = np.zeros(nidx, np.int16)
            lc_full = np.full(nidx, 999.0, np.float32)
            off = 0
            ptr = 0
            for g in range(NGRP):
                ncg = int(nch_grp_pad[h, g])
                k = int((gg == g).sum())
                idx_full[off:off + k] = gi[ptr:ptr + k]
                lc_full[off:off + k] = gl[ptr:ptr + k]
                ptr += k
                off += ncg * 128
            nch = nch_s[h]
            colv = lc_full.reshape(nch, 128).T.copy()        # [128, nch]
            tabs[h] = (_wrap_idx(idx_full, nch), colv)
        per_core.append(tabs)
    return dinv, per_core, nch_grp_pad, nch_s


def _build(nch_grp_pad, nch_s, skip_coll=False, ag_frac=1):
    nc = bacc.Bacc("TRN2", target_bir_lowering=False, debug=False,
                   num_devices=NCORES, num_swdge_queues=NQUEUES)
    # inputs
    t_x = nc.dram_tensor("x_shard", [PPAD, F], fp32, kind="ExternalInput")
    t_w1 = nc.dram_tensor("w1", [F, H], fp32, kind="ExternalInput")
    t_w2f = nc.dram_tensor("w2f", [H, FO], fp16, kind="ExternalInput")
    t_ident = nc.dram_tensor("ident", [128, 128], fp32, kind="ExternalInput")
    t_iota = nc.dram_tensor("iota8", [128, SB * 128], fp16, kind="ExternalInput")
    t_dpp = nc.dram_tensor("dinv_pp", [128, NGRP], fp32, kind="ExternalInput")
    t_dbc = nc.dram_tensor("dinv_bc", [128, PPAD], fp32, kind="ExternalInput")
    t_b1 = nc.dram_tensor("b1_pp", [128, 1], fp32, kind="ExternalInput")
    t_b2 = nc.dram_tensor("b2_bc", [128, FO], fp32, kind="ExternalInput")
    t_idx = [nc.dram_tensor(f"idx{h}", [128, nch_s[h] * 8], i16, kind="ExternalInput")
             for h in (0, 1)]
    t_col = [nc.dram_tensor(f"col{h}", [128, nch_s[h]], fp32, kind="ExternalInput")
             for h in (0, 1)]
    t_out = nc.dram_tensor("out_shard", [PPAD, FO], fp32, kind="ExternalOutput")

    with TileContext(nc, num_cores=NCORES) as tc, ExitStack() as ex:
        nc.gpsimd.load_library(mlp)
        cst = ex.enter_context(tc.tile_pool(name="cst", bufs=1))
        xb = ex.enter_context(tc.tile_pool(name="xb", bufs=3))
        gp = ex.enter_context(tc.tile_pool(name="gp", bufs=6))
        sp = ex.enter_context(tc.tile_pool(name="sp", bufs=4))
        ep = ex.enter_context(tc.tile_pool(name="ep", bufs=3))
        pst = ex.enter_context(tc.tile_pool(name="pst", bufs=2, space="PSUM"))
        psa = ex.enter_context(tc.tile_pool(name="psa", bufs=2, space="PSUM"))
        dram = ex.enter_context(tc.tile_pool(name="dram", bufs=1, space="DRAM"))

        # constants
        w1_sb = cst.tile([F, H], fp32); nc.sync.dma_start(w1_sb[:], t_w1[:])
        w2_sb = cst.tile([H, FO], fp16); nc.sync.dma_start(w2_sb[:], t_w2f[:])
        id_sb = cst.tile([128, 128], fp32); nc.sync.dma_start(id_sb[:], t_ident[:])
        io_sb = cst.tile([128, SB * 128], fp16); nc.sync.dma_start(io_sb[:], t_iota[:])
        dpp_sb = cst.tile([128, NGRP], fp32); nc.sync.dma_start(dpp_sb[:], t_dpp[:])
        dbc_sb = cst.tile([128, PPAD], fp32); nc.sync.dma_start(dbc_sb[:], t_dbc[:])
        b1_sb = cst.tile([128, 1], fp32); nc.sync.dma_start(b1_sb[:], t_b1[:])
        b2_sb = cst.tile([128, FO], fp32); nc.sync.dma_start(b2_sb[:], t_b2[:])
        idx_sb = []
        col_sb = []
        for h in (0, 1):
            it = cst.tile([128, nch_s[h] * 8], i16, tag=f"idx{h}")
            nc.sync.dma_start(it[:], t_idx[h][:])
            idx_sb.append(it)
            ct = cst.tile([128, nch_s[h]], fp32, tag=f"col{h}")
            nc.sync.dma_start(ct[:], t_col[h][:])
            col_sb.append(ct)

        # DRAM bounce buffers
        ag1_in = dram.tile([PPAD, H], fp16)
        g1_full = dram.tile([TBL, H], fp16)
        ag2_in = dram.tile([PPAD, H], fp16)   # g2 padded to 128 wide
        g2_full = dram.tile([TBL, H], fp16)

        # ---- phase 0: g1 = dinv * (x @ W1), write own shard ----
        for t in range(NGRP):
            x_t = xb.tile([128, F], fp32, tag="x")
            nc.sync.dma_start(x_t[:], t_x[t * 128:(t + 1) * 128, :])
            xT = pst.tile([128, 128], fp32, tag="xT")
            nc.tensor.transpose(xT[:], x_t[:], id_sb[:])
            xT_sb = xb.tile([128, 128], fp32, tag="xTs")
            nc.vector.tensor_copy(xT_sb[:], xT[:])
            z = psa.tile([128, H], fp32, tag="z")
            nc.tensor.matmul(z[:], xT_sb[:], w1_sb[:], start=True, stop=True)
            g1_t = xb.tile([128, H], fp16, tag="g1")
            nc.vector.tensor_scalar(g1_t[:], z[:], dpp_sb[:, t:t + 1], None,
                                    op0=mybir.AluOpType.mult)
            nc.sync.dma_start(ag1_in[t * 128:(t + 1) * 128, :], g1_t[:])

        if skip_coll:
            nc.sync.dma_start(g1_full[0:PPAD, :], ag1_in[:])
        elif ag_frac > 1:
            q = PPAD // ag_frac
            nc.gpsimd.collective_compute(
                "AllGather", mybir.AluOpType.bypass,
                replica_groups=[list(range(NCORES))],
                ins=[ag1_in[0:q, :]], outs=[g1_full[0:NCORES * q, :]])
        else:
            nc.gpsimd.collective_compute(
                "AllGather", mybir.AluOpType.bypass,
                replica_groups=[list(range(NCORES))],
                ins=[ag1_in.opt()], outs=[g1_full.opt()])

        # ---- sparse layer pass (shared structure for both layers) ----
        def sparse_pass(src_full, layer):
            srcs = [src_full[0:HALF, :], src_full[HALF:TBL, :]]
            # chunk plan: per group, (half, chunk-slot) list; stream positions
            goff = [[0] + list(np.cumsum(nch_grp_pad[h]).astype(int))
                    for h in (0, 1)]
            gt = {}   # gather tiles per (h, superchunk)
            st = {}   # S tiles per (h, sbatch)

            def get_g(h, sc):
                if (h, sc) not in gt:
                    gtile = gp.tile([128, GB, H], fp16, tag="g")
                    nidx = GB * 128
                    nc.gpsimd.dma_gather(
                        gtile[:], srcs[h], idx_sb[h][:, sc * (GB * 8):(sc + 1) * (GB * 8)],
                        nidx, nidx, H, single_packet=SINGLE_PACKET,
                        queue_num=(h * 7 + sc) % NQUEUES)
                    gt[(h, sc)] = gtile
                return gt[(h, sc)]

            def get_s(h, sb):
                if (h, sb) not in st:
                    stile = sp.tile([128, SB, 128], fp16, tag="s")
                    colap = col_sb[h][:, sb * SB:(sb + 1) * SB]
                    colap = colap[:, :, None].broadcast_to([128, SB, 128])
                    nc.vector.tensor_tensor(
                        stile[:], io_sb[:].rearrange("p (a b) -> p a b", a=SB),
                        colap, op=mybir.AluOpType.is_equal)
                    st[(h, sb)] = stile
                return st[(h, sb)]

            for g in range(NGRP):
                acc = psa.tile([128, 128], fp32, tag="acc")
                plan = []
                for h in (0, 1):
                    plan += [(h, k) for k in range(goff[h][g], goff[h][g + 1])]
                for j, (h, k) in enumerate(plan):
                    gtile = get_g(h, k // GB)
                    stile = get_s(h, k // SB)
                    s_ap = stile[:, k % SB, :]
                    g_ap = gtile[:, k % GB, :]
                    first, last = j == 0, j == len(plan) - 1
                    if layer == 1:
                        nc.tensor.matmul(acc[:], g_ap, s_ap,
                                         start=first, stop=last)
                    else:
                        nc.tensor.matmul(acc[:, 0:FO], s_ap, g_ap[:, 0:FO],
                                         start=first, stop=last)
                yield g, acc

        # ---- layer 1: h1 = relu(dinv*acc + b1) ; g2 = dinv*(h1 @ W2) ----
        for g, acc in sparse_pass(g1_full, 1):
            tmp = ep.tile([128, 128], fp32, tag="tmp")
            nc.vector.tensor_tensor(tmp[:], acc[:],
                                    dbc_sb[:, g * 128:(g + 1) * 128],
                                    op=mybir.AluOpType.mult)
            h1 = ep.tile([128, 128], fp16, tag="h1")
            nc.scalar.activation(h1[:], tmp[:], mybir.ActivationFunctionType.Relu,
                                 bias=b1_sb[:, 0:1])
            y2 = pst.tile([128, FO], fp32, tag="y2")
            nc.tensor.matmul(y2[:], h1[:], w2_sb[:], start=True, stop=True)
            g2_t = ep.tile([128, H], fp16, tag="g2")
            nc.vector.tensor_scalar(g2_t[:, 0:FO], y2[:], dpp_sb[:, g:g + 1], None,
                                    op0=mybir.AluOpType.mult)
            nc.gpsimd.memset(g2_t[:, FO:H], 0.0)
            nc.sync.dma_start(ag2_in[g * 128:(g + 1) * 128, :], g2_t[:])

        if skip_coll:
            nc.sync.dma_start(g2_full[0:PPAD, :], ag2_in[:])
        elif ag_frac > 1:
            q = PPAD // ag_frac
            nc.gpsimd.collective_compute(
                "AllGather", mybir.AluOpType.bypass,
                replica_groups=[list(range(NCORES))],
                ins=[ag2_in[0:q, :]], outs=[g2_full[0:NCORES * q, :]])
        else:
            nc.gpsimd.collective_compute(
                "AllGather", mybir.AluOpType.bypass,
                replica_groups=[list(range(NCORES))],
                ins=[ag2_in.opt()], outs=[g2_full.opt()])

        # ---- layer 2: out = dinv*acc + b2 ----
        for g, acc in sparse_pass(g2_full, 2):
            o1 = ep.tile([128, FO], fp32, tag="o1")
            nc.vector.tensor_scalar(o1[:], acc[:, 0:FO], dpp_sb[:, g:g + 1], None,
                                    op0=mybir.AluOpType.mult)
            o2 = ep.tile([128, FO], fp32, tag="o2")
            nc.vector.tensor_tensor(o2[:], o1[:], b2_sb[:], op=mybir.AluOpType.add)
            nc.sync.dma_start(t_out[g * 128:(g + 1) * 128, :], o2[:])

    nc.compile()
    return nc


SKIP_COLL = False
AG_FRAC = 1
SINGLE_PACKET = True
NQUEUES = 1


def kernel(x, edge_index, W1, b1, W2, b2):
    x = np.asarray(x, np.float32)
    W1 = np.asarray(W1, np.float32)
    W2 = np.asarray(W2, np.float32)
    b1 = np.asarray(b1, np.float32)
    b2 = np.asarray(b2, np.float32)

    dinv, per_core, nch_grp_pad, nch_s = _prep(edge_index)
    nc = _build(nch_grp_pad, nch_s, skip_coll=SKIP_COLL, ag_frac=AG_FRAC)

    iota8 = np.tile(np.arange(128, dtype=np.float16)[None, :], (128, SB))
    ident = np.eye(128, dtype=np.float32)
    b1_pp = b1.reshape(128, 1)
    b2_bc = np.tile(b2[None, :], (128, 1)).astype(np.float32)

    in_maps = []
    for c in range(NCORES):
        own = dinv[c * P:(c + 1) * P]
        own_pad = np.zeros(PPAD, np.float32)
        own_pad[:P] = own
        xs = np.zeros((PPAD, F), np.float32)
        xs[:P] = x[c * P:(c + 1) * P]
        m = {
            "x_shard": xs,
            "w1": W1,
            "w2f": W2.astype(np.float16),
            "ident": ident,
            "iota8": iota8,
            "dinv_pp": own_pad.reshape(NGRP, 128).T.copy(),
            "dinv_bc": np.tile(own_pad[None, :], (128, 1)),
            "b1_pp": b1_pp,
            "b2_bc": b2_bc,
        }
        for h in (0, 1):
            idxw, colv = per_core[c][h]
            m[f"idx{h}"] = idxw
            m[f"col{h}"] = colv
        in_maps.append(m)

    global LAST_NC, LAST_IN_MAPS
    LAST_NC, LAST_IN_MAPS = nc, in_maps
    res = run_bass_kernel_spmd(nc, in_maps, core_ids=list(range(NCORES)))
    out = np.concatenate(
        [res.results[c]["out_shard"][:P] for c in range(NCORES)], axis=0)
    return out.astype(np.float32)

